# revision 1
# baseline (speedup 1.0000x reference)
"""Trainium2 Bass kernel for BiFormer-style sparse window attention routing
(nn_BA_28784870818378), SPMD across 8 NeuronCores.

Host contract: kernel(x, w_qkv, b_qkv) takes the FULL inputs
(x (2,192,256,256) f32, w_qkv (192,576) f32, b_qkv (576,) f32) and returns
the FULL output (2, 1024, 4, 64, 192) f32.

Sharding: core c handles batch c//4 and query-window quarter c%4. Every core
computes the full-batch per-pixel v projection (in fp16; means/routing in
exact fp32) plus the full routing, then gathers only its quarter's selected
window blocks via indirect DMA. Host concatenates the 8 partial outputs and
casts fp16 -> fp32.
"""

import numpy as np

import concourse.bass as bass
import concourse.mybir as mybir
from concourse.bass import IndirectOffsetOnAxis
from concourse.tile import TileContext
from concourse.vector_clock import ScopedClock


_orig_commit_and_lower = TileContext._commit_and_lower


def _split_commit_and_lower(self, inst, original_block, old_bb_map, bb_to_exit_bb):
    si = inst.sync_info
    if si is not None and si.on_wait is not None and len(si.on_wait) > 1:
        waits = list(si.on_wait)
        updates = list(si.on_update) if si.on_update else []
        inst.sync_info = mybir.SyncInfo(on_wait=[waits[-1]], on_update=updates)
        for w in waits[:-1]:
            nop = mybir.InstNoOp(
                name=self.nc.get_next_instruction_name(),
                engine=inst.engine,
                ins=[],
                outs=[],
                sync_info=mybir.SyncInfo(on_wait=[w], on_update=[]),
                bass_nofuse=True,
            )
            _orig_commit_and_lower(self, nop, original_block, old_bb_map, bb_to_exit_bb)
    return _orig_commit_and_lower(self, inst, original_block, old_bb_map, bb_to_exit_bb)


def _patched_drain_and_barrier(self, tick_clock, wait_clock):
    nop0 = self.nc.sync.nop(nofuse=True, hint="drain_waits")
    wait_clock.add_sem_waits(nop0.ins, ScopedClock({None: tick_clock.global_clock}))
    si = nop0.ins.sync_info
    waits = list(si.on_wait) if si is not None and si.on_wait else []
    if len(waits) > 1:
        nop0.ins.sync_info = mybir.SyncInfo(on_wait=[waits[0]], on_update=[])
        for w in waits[1:]:
            nopi = self.nc.sync.nop(nofuse=True, hint="drain_waits")
            nopi.ins.sync_info = mybir.SyncInfo(on_wait=[w], on_update=[])
    self.nc.sync.drain()

    self.nc.all_engine_barrier()
    assert self.sems is not None
    popped = self.nc._tile_sem_poison_stack.pop()
    assert popped is self._sem_poison
    self.nc.clear_and_free_semaphores(list(self.sems.allocated().values()))
    self.nc.all_engine_barrier()


def _apply_walrus_workarounds():
    TileContext._commit_and_lower = _split_commit_and_lower
    TileContext._drain_and_barrier = _patched_drain_and_barrier


F32 = mybir.dt.float32
F16 = mybir.dt.float16
U32 = mybir.dt.uint32

C = 192            # channels
H = W = 256
WIN = 8
NH = NW = H // WIN  # 32
NWIN = NH * NW      # 1024 windows per batch
SHW = WIN * WIN     # 64 pixels per window
TOPK = 4
D = 192            # v dim
QK = 192
BLK = SHW * D      # 12288 elements per gathered block
SCALE = QK ** -0.5


_apply_walrus_workarounds()


def build_nc():
    nc = bass.Bass("TRN2")
    x = nc.dram_tensor("x", [C, H * W], F32, kind="ExternalInput")
    wqkv = nc.dram_tensor("wqkv", [C, 576], F32, kind="ExternalInput")
    bqkv = nc.dram_tensor("bqkv", [576, 1], F32, kind="ExternalInput")
    ssel = nc.dram_tensor("ssel", [NWIN, 1], U32, kind="ExternalInput")
    out = nc.dram_tensor("out", [NWIN, BLK], F16, kind="ExternalOutput")

    vpix = nc.dram_tensor("vpix", [NWIN, BLK], F16, kind="Internal")
    idxf = nc.dram_tensor("idxf", [NWIN * TOPK, 1], U32, kind="Internal")

    with TileContext(nc) as tc:
        with (
            tc.tile_pool(name="const", bufs=1) as cp,
            tc.tile_pool(name="slab", bufs=1) as sp,
            tc.tile_pool(name="psv", bufs=3, space="PSUM") as ppv,
            tc.tile_pool(name="psp", bufs=2, space="PSUM") as ppp,
            tc.tile_pool(name="psl", bufs=3, space="PSUM") as ppl,
            tc.tile_pool(name="gat", bufs=2) as gp,
            tc.tile_pool(name="p3s", bufs=2) as p3,
        ):
            # ---- constants: weights + biases --------------------------------
            wa = cp.tile([128, 576], F32, tag="wa")
            wb = cp.tile([65, 576], F32, tag="wb")  # 64 ch + bias row
            nc.sync.dma_start(out=wa[:], in_=wqkv[0:128, :])
            nc.sync.dma_start(out=wb[0:64, :], in_=wqkv[128:192, :])
            nc.sync.dma_start(out=wb[64:65, :], in_=bqkv[:, 0:1].rearrange("d one -> (one) d"))

            # per-partition bias tiles for q/k projections
            bqa = cp.tile([128, 1], F32, tag="bqa")
            bqb = cp.tile([64, 1], F32, tag="bqb")
            bka = cp.tile([128, 1], F32, tag="bka")
            bkb = cp.tile([64, 1], F32, tag="bkb")
            nc.sync.dma_start(out=bqa[:], in_=bqkv[0:128, :])
            nc.sync.dma_start(out=bqb[:], in_=bqkv[128:192, :])
            nc.sync.dma_start(out=bka[:], in_=bqkv[192:320, :])
            nc.sync.dma_start(out=bkb[:], in_=bqkv[320:384, :])
            # q bias must be pre-scaled by SCALE (logits use scale*q_win)
            nc.scalar.mul(bqa[:], bqa[:], SCALE)
            nc.scalar.mul(bqb[:], bqb[:], SCALE)

            # fp16 copies of the v-projection weights (+ bias row in wb16)
            wv16a = cp.tile([128, 192], F16, tag="wv16a")
            wv16b = cp.tile([65, 192], F16, tag="wv16b")
            nc.vector.tensor_copy(out=wv16a[:], in_=wa[:, 384:576])
            nc.vector.tensor_copy(out=wv16b[:], in_=wb[:, 384:576])

            # window-mean accumulators (raw sums; /64 folded into proj scale)
            xbar_a = cp.tile([128, NWIN], F32, tag="xbar_a")
            xbar_b = cp.tile([64, NWIN], F32, tag="xbar_b")

            # ---- phase 1: slabs -> means + v --------------------------------
            NBUF = 3
            xa_t = [sp.tile([128, 2048], F32, tag=f"xa{i}", name=f"xa{i}") for i in range(NBUF)]
            xb_t = [sp.tile([65, 2048], F32, tag=f"xb{i}", name=f"xb{i}") for i in range(NBUF)]
            xa16_t = [sp.tile([128, 2048], F16, tag=f"xa16_{i}", name=f"xa16_{i}") for i in range(NBUF)]
            xb16_t = [sp.tile([65, 2048], F16, tag=f"xb16_{i}", name=f"xb16_{i}") for i in range(NBUF)]
            st_t = [sp.tile([128, 1536], F16, tag=f"st{i}", name=f"st{i}") for i in range(NBUF)]
            for i in range(NBUF):
                nc.vector.memset(xb16_t[i][64:65, :], 1.0)

            # v DRAM write view: staging tiles hold 16 full windows
            # (partitions = (e, pix), free = (j, ch); window = 32*nh+16*e+8*s+j)
            # -> per (staging, e): 3-dim AP [[192,64],[12288,8],[1,192]].
            vw = vpix[:].rearrange(
                "(q j) (pix c) -> q pix j c", j=8, c=D,
            )  # [128, 64, 8, 192]

            # ---- phase 2 state: projections, logits, top-4 ------------------
            # emitted in halves: window columns 0..511 depend only on slabs
            # 0..15, so half-0 projections/logits hide under phase 1.
            qta = cp.tile([128, NWIN], F32, tag="qta")
            qtb = cp.tile([64, NWIN], F32, tag="qtb")
            kta = cp.tile([128, NWIN], F32, tag="kta")
            ktb = cp.tile([64, NWIN], F32, tag="ktb")
            lg_t = [cp.tile([128, NWIN], F32, tag=f"lg{i}", name=f"lg{i}")
                    for i in range(8)]
            COPY = mybir.ActivationFunctionType.Identity
            idv = idxf[:].rearrange("(q p t) one -> q p (t one)", p=128, t=TOPK)

            def emit_proj(qc):
                # projection for window-quarter qc (cols 256qc..256qc+255),
                # ready as soon as slab 8*qc+7 has been reduced
                nsl = slice(256 * qc, 256 * (qc + 1))
                for col0, (ta, tb), sc, (ba, bb) in (
                    (0, (qta, qtb), SCALE / SHW, (bqa, bqb)),
                    (192, (kta, ktb), 1.0 / SHW, (bka, bkb)),
                ):
                    for (t_out, d0, dn) in ((ta, 0, 128), (tb, 128, 64)):
                        ps = ppp.tile([dn, 256], F32, tag="pproj", name="ps_proj")
                        nc.tensor.matmul(
                            ps[:], lhsT=wa[:, col0 + d0 : col0 + d0 + dn],
                            rhs=xbar_a[:, nsl], start=True, stop=False,
                        )
                        nc.tensor.matmul(
                            ps[:], lhsT=wb[0:64, col0 + d0 : col0 + d0 + dn],
                            rhs=xbar_b[:, nsl], start=False, stop=True,
                        )
                        nc.scalar.activation(
                            out=t_out[:, nsl], in_=ps[:], func=COPY,
                            bias=ba[:, 0:1] if dn == 128 else bb[:, 0:1], scale=sc,
                        )

            # per-(row-tile, quarter) top-8 partials, computed as soon as a
            # quarter's logits land; the tail only combines + indexes
            qm_t = [cp.tile([128, 32], F32, tag=f"qm{i}", name=f"qm{i}")
                    for i in range(8)]

            def emit_logits(nt, mq):
                # logits row-tile nt vs key-quarter mq
                ps = ppl.tile([128, 256], F32, tag="plog", name="ps_log")
                msl = slice(256 * mq, 256 * (mq + 1))
                nc.tensor.matmul(
                    ps[:], lhsT=qta[:, 128 * nt : 128 * (nt + 1)],
                    rhs=kta[:, msl], start=True, stop=False,
                )
                nc.tensor.matmul(
                    ps[:], lhsT=qtb[:, 128 * nt : 128 * (nt + 1)],
                    rhs=ktb[:, msl], start=False, stop=True,
                )
                if (nt + mq) % 2 == 0:
                    nc.vector.tensor_copy(out=lg_t[nt][:, msl], in_=ps[:])
                else:
                    nc.scalar.copy(out=lg_t[nt][:, msl], in_=ps[:])
                nc.vector.max(out=qm_t[nt][:, 8 * mq : 8 * (mq + 1)], in_=lg_t[nt][:, msl])

            def emit_max(nt):
                lg = lg_t[nt]
                mx8 = p3.tile([128, 8], F32, tag="mx8", name="mx8")
                mi8 = p3.tile([128, 8], U32, tag="mi8", name="mi8")
                # top-8 of the 4 quarter-top-8s == global top-8
                nc.vector.max(out=mx8[:], in_=qm_t[nt][:])
                nc.vector.max_index(out=mi8[:], in_max=mx8[:], in_values=lg[:])
                nc.sync.dma_start(out=idv[nt], in_=mi8[:, 0:TOPK])

            for nh in range(NH):
                xa = xa_t[nh % NBUF]
                xb = xb_t[nh % NBUF]
                xa16 = xa16_t[nh % NBUF]
                xb16 = xb16_t[nh % NBUF]
                nc.scalar.dma_start(out=xa[:], in_=x[0:128, 2048 * nh : 2048 * (nh + 1)])
                nc.scalar.dma_start(out=xb[0:64, :], in_=x[128:192, 2048 * nh : 2048 * (nh + 1)])
                # gpsimd (otherwise idle in phase 1) casts f32->f16 AND
                # reorders to window-major: fp16 column b*128+(e,dh,dw) =
                # pixel (dh,dw) of window nw = 16*e + b.
                xa_wm = xa[:].rearrange("p (dh e b dw) -> p b e dh dw", dh=8, e=2, b=16, dw=8)
                xb_wm = xb[:].rearrange("p (dh e b dw) -> p b e dh dw", dh=8, e=2, b=16, dw=8)
                nc.gpsimd.tensor_copy(out=xa16[:], in_=xa_wm)
                nc.gpsimd.tensor_copy(out=xb16[0:64, :], in_=xb_wm[0:64])
                xar = xa[:].rearrange("p (dh nw dw) -> p nw dh dw", dh=8, nw=32, dw=8)
                xbr = xb[:].rearrange("p (dh nw dw) -> p nw dh dw", dh=8, nw=32, dw=8)
                nc.vector.reduce_sum(
                    out=xbar_a[:, 32 * nh : 32 * (nh + 1)], in_=xar,
                    axis=mybir.AxisListType.XY,
                )
                nc.vector.reduce_sum(
                    out=xbar_b[:, 32 * nh : 32 * (nh + 1)], in_=xbr[0:64],
                    axis=mybir.AxisListType.XY,
                )
                for s in range(2):  # staging tiles: blocks 8s..8s+7
                    st = st_t[(2 * nh + s) % NBUF]
                    for jp in range(4):
                        ps = ppv.tile([128, 384], F32, tag="vps")
                        for sub in range(2):
                            b = 8 * s + 2 * jp + sub
                            o = ps[:, 192 * sub : 192 * (sub + 1)]
                            nc.tensor.matmul(
                                o, lhsT=xa16[:, 128 * b : 128 * (b + 1)],
                                rhs=wv16a[:], start=True, stop=False,
                            )
                            nc.tensor.matmul(
                                o, lhsT=xb16[0:65, 128 * b : 128 * (b + 1)],
                                rhs=wv16b[:], start=False, stop=True,
                            )
                        # psum f32 -> staging f16; alternate DVE/ACT, but
                        # keep DVE clear near the end so the last window-mean
                        # reduces (which gate phase 2) are not queued behind
                        # evac copies
                        if jp % 2 == 0 and nh < 26:
                            nc.vector.tensor_copy(
                                out=st[:, 384 * jp : 384 * (jp + 1)], in_=ps[:]
                            )
                        else:
                            nc.scalar.copy(
                                out=st[:, 384 * jp : 384 * (jp + 1)], in_=ps[:]
                            )
                    # two window-contiguous DMAs: e=0 -> windows 32nh+8s+j,
                    # e=1 -> windows 32nh+16+8s+j (j in 0..8). On the SP ring
                    # (idle in phase 1) so their waits never head-block x loads
                    # or ACT evac copies.
                    for e in range(2):
                        q = 4 * nh + 2 * e + s
                        nc.sync.dma_start(out=vw[q], in_=st[64 * e : 64 * (e + 1), :])

                if nh == 15:
                    # q/k quarters 0-1 + all logits touching only them
                    emit_proj(0)
                    emit_proj(1)
                    for nt in range(4):
                        emit_logits(nt, 0)
                        emit_logits(nt, 1)
                elif nh == 23:
                    # quarter 2: rows 512-767 (nt 4,5) and key cols 512-767
                    emit_proj(2)
                    for nt in range(4):
                        emit_logits(nt, 2)
                    for nt in (4, 5):
                        emit_logits(nt, 0)
                        emit_logits(nt, 1)
                        emit_logits(nt, 2)

            emit_proj(3)
            for nt in range(6):
                emit_logits(nt, 3)
                emit_max(nt)
            for nt in (6, 7):
                for mq in range(4):
                    emit_logits(nt, mq)
                emit_max(nt)

            # ---- phase 3 gather: run the window-id resolution + v gathers ----
            igs = []
            for g in range(8):
                sst = p3.tile([128, 1], U32, tag=f"sst{g}", name=f"sst{g}")
                nc.sync.dma_start(out=sst[:], in_=ssel[128 * g : 128 * (g + 1), :])
                ig = p3.tile([128, 1], U32, tag=f"ig{g}", name=f"ig{g}")
                nc.gpsimd.indirect_dma_start(
                    out=ig[:], out_offset=None, in_=idxf[:],
                    in_offset=IndirectOffsetOnAxis(ap=sst[:, 0:1], axis=0),
                )
                igs.append(ig)
                gt = gp.tile([128, BLK], F16, tag="gt")
                nc.gpsimd.indirect_dma_start(
                    out=gt[:], out_offset=None, in_=vpix[:],
                    in_offset=IndirectOffsetOnAxis(ap=ig[:, 0:1], axis=0),
                )
                nc.scalar.dma_start(out=out[128 * g : 128 * (g + 1), :], in_=gt[:])

    return nc


def make_in_maps(x_full, w_qkv, b_qkv):
    """x_full (2, 192, 256, 256) -> per-core input dicts."""
    ins = []
    for core in range(8):
        b = core // 4
        q = core % 4
        ins.append(
            {
                "x": np.ascontiguousarray(x_full[b].reshape(C, H * W)),
                "wqkv": np.ascontiguousarray(w_qkv),
                "bqkv": np.ascontiguousarray(b_qkv.reshape(576, 1)),
                "ssel": np.arange(1024 * q, 1024 * (q + 1), dtype=np.uint32).reshape(
                    NWIN, 1
                ),
            }
        )
    return ins


def assemble(results):
    """per-core 'out' (1024, 12288) -> (2, 1024, 4, 64, 192)."""
    full = np.empty((2, NWIN, TOPK, SHW, D), dtype=np.float32)
    for core in range(8):
        b = core // 4
        q = core % 4
        r = results[core]["out"].astype(np.float32).reshape(256, TOPK, SHW, D)
        full[b, 256 * q : 256 * (q + 1)] = r
    return full


_NC_CACHE = None


def _get_nc():
    global _NC_CACHE
    if _NC_CACHE is None:
        _NC_CACHE = build_nc()
    return _NC_CACHE


def kernel(x, w_qkv, b_qkv):
    from concourse.bass_utils import run_bass_kernel_spmd

    x = np.ascontiguousarray(np.asarray(x, dtype=np.float32))
    w_qkv = np.ascontiguousarray(np.asarray(w_qkv, dtype=np.float32))
    b_qkv = np.ascontiguousarray(np.asarray(b_qkv, dtype=np.float32))

    nc = _get_nc()
    in_maps = make_in_maps(x, w_qkv, b_qkv)
    res = run_bass_kernel_spmd(nc, in_maps, core_ids=list(range(8)))
    return assemble(res.results)



# revision 3
# speedup vs baseline: 1.0501x; 1.0501x over previous
"""Trainium2 Bass kernel for BiFormer-style sparse window attention routing
(nn_BA_28784870818378), SPMD across 8 NeuronCores.

Host contract: kernel(x, w_qkv, b_qkv) takes the FULL inputs
(x (2,192,256,256) f32, w_qkv (192,576) f32, b_qkv (576,) f32) and returns
the FULL output (2, 1024, 4, 64, 192) f32.

Sharding: core c handles batch c//4 and query-window quarter c%4. Every core
computes the full-batch per-pixel v projection (in fp16; means/routing in
exact fp32) plus the full routing, then gathers only its quarter's selected
window blocks via indirect DMA. Host concatenates the 8 partial outputs and
casts fp16 -> fp32.
"""

import numpy as np

import concourse.bass as bass
import concourse.mybir as mybir
from concourse.bass import IndirectOffsetOnAxis
from concourse.tile import TileContext
from concourse.vector_clock import ScopedClock


_orig_commit_and_lower = TileContext._commit_and_lower


def _split_commit_and_lower(self, inst, original_block, old_bb_map, bb_to_exit_bb):
    si = inst.sync_info
    if si is not None and si.on_wait is not None and len(si.on_wait) > 1:
        waits = list(si.on_wait)
        updates = list(si.on_update) if si.on_update else []
        inst.sync_info = mybir.SyncInfo(on_wait=[waits[-1]], on_update=updates)
        for w in waits[:-1]:
            nop = mybir.InstNoOp(
                name=self.nc.get_next_instruction_name(),
                engine=inst.engine,
                ins=[],
                outs=[],
                sync_info=mybir.SyncInfo(on_wait=[w], on_update=[]),
                bass_nofuse=True,
            )
            _orig_commit_and_lower(self, nop, original_block, old_bb_map, bb_to_exit_bb)
    return _orig_commit_and_lower(self, inst, original_block, old_bb_map, bb_to_exit_bb)


def _patched_drain_and_barrier(self, tick_clock, wait_clock):
    nop0 = self.nc.sync.nop(nofuse=True, hint="drain_waits")
    wait_clock.add_sem_waits(nop0.ins, ScopedClock({None: tick_clock.global_clock}))
    si = nop0.ins.sync_info
    waits = list(si.on_wait) if si is not None and si.on_wait else []
    if len(waits) > 1:
        nop0.ins.sync_info = mybir.SyncInfo(on_wait=[waits[0]], on_update=[])
        for w in waits[1:]:
            nopi = self.nc.sync.nop(nofuse=True, hint="drain_waits")
            nopi.ins.sync_info = mybir.SyncInfo(on_wait=[w], on_update=[])
    self.nc.sync.drain()

    self.nc.all_engine_barrier()
    assert self.sems is not None
    popped = self.nc._tile_sem_poison_stack.pop()
    assert popped is self._sem_poison
    self.nc.clear_and_free_semaphores(list(self.sems.allocated().values()))
    self.nc.all_engine_barrier()


def _apply_walrus_workarounds():
    TileContext._commit_and_lower = _split_commit_and_lower
    TileContext._drain_and_barrier = _patched_drain_and_barrier


F32 = mybir.dt.float32
F16 = mybir.dt.float16
U32 = mybir.dt.uint32

C = 192            # channels
H = W = 256
WIN = 8
NH = NW = H // WIN  # 32
NWIN = NH * NW      # 1024 windows per batch
SHW = WIN * WIN     # 64 pixels per window
TOPK = 4
D = 192            # v dim
QK = 192
BLK = SHW * D      # 12288 elements per gathered block
SCALE = QK ** -0.5


_apply_walrus_workarounds()


def build_nc():
    nc = bass.Bass("TRN2")
    x = nc.dram_tensor("x", [C, H * W], F32, kind="ExternalInput")
    wqkv = nc.dram_tensor("wqkv", [C, 576], F32, kind="ExternalInput")
    bqkv = nc.dram_tensor("bqkv", [576, 1], F32, kind="ExternalInput")
    ssel = nc.dram_tensor("ssel", [NWIN, 1], U32, kind="ExternalInput")
    out = nc.dram_tensor("out", [NWIN, BLK], F16, kind="ExternalOutput")

    vpix = nc.dram_tensor("vpix", [NWIN, BLK], F16, kind="Internal")
    idxf = nc.dram_tensor("idxf", [NWIN * TOPK, 1], U32, kind="Internal")

    with TileContext(nc) as tc:
        with (
            tc.tile_pool(name="const", bufs=1) as cp,
            tc.tile_pool(name="slab", bufs=1) as sp,
            tc.tile_pool(name="psv", bufs=3, space="PSUM") as ppv,
            tc.tile_pool(name="psp", bufs=2, space="PSUM") as ppp,
            tc.tile_pool(name="psl", bufs=3, space="PSUM") as ppl,
            tc.tile_pool(name="gat", bufs=2) as gp,
            tc.tile_pool(name="p3s", bufs=2) as p3,
        ):
            # ---- constants: weights + biases --------------------------------
            wa = cp.tile([128, 576], F32, tag="wa")
            wb = cp.tile([65, 576], F32, tag="wb")  # 64 ch + bias row
            nc.sync.dma_start(out=wa[:], in_=wqkv[0:128, :])
            nc.sync.dma_start(out=wb[0:64, :], in_=wqkv[128:192, :])
            nc.sync.dma_start(out=wb[64:65, :], in_=bqkv[:, 0:1].rearrange("d one -> (one) d"))

            # per-partition bias tiles for q/k projections
            bqa = cp.tile([128, 1], F32, tag="bqa")
            bqb = cp.tile([64, 1], F32, tag="bqb")
            bka = cp.tile([128, 1], F32, tag="bka")
            bkb = cp.tile([64, 1], F32, tag="bkb")
            nc.sync.dma_start(out=bqa[:], in_=bqkv[0:128, :])
            nc.sync.dma_start(out=bqb[:], in_=bqkv[128:192, :])
            nc.sync.dma_start(out=bka[:], in_=bqkv[192:320, :])
            nc.sync.dma_start(out=bkb[:], in_=bqkv[320:384, :])
            # q bias must be pre-scaled by SCALE (logits use scale*q_win)
            nc.scalar.mul(bqa[:], bqa[:], SCALE)
            nc.scalar.mul(bqb[:], bqb[:], SCALE)

            # fp16 copies of the v-projection weights (+ bias row in wb16)
            wv16a = cp.tile([128, 192], F16, tag="wv16a")
            wv16b = cp.tile([65, 192], F16, tag="wv16b")
            nc.vector.tensor_copy(out=wv16a[:], in_=wa[:, 384:576])
            nc.vector.tensor_copy(out=wv16b[:], in_=wb[:, 384:576])

            # window-mean accumulators (raw sums; /64 folded into proj scale)
            xbar_a = cp.tile([128, NWIN], F32, tag="xbar_a")
            xbar_b = cp.tile([64, NWIN], F32, tag="xbar_b")

            # ---- phase 1: slabs -> means + v --------------------------------
            NBUF = 3
            xa_t = [sp.tile([128, 2048], F32, tag=f"xa{i}", name=f"xa{i}") for i in range(NBUF)]
            xb_t = [sp.tile([65, 2048], F32, tag=f"xb{i}", name=f"xb{i}") for i in range(NBUF)]
            xa16_t = [sp.tile([128, 2048], F16, tag=f"xa16_{i}", name=f"xa16_{i}") for i in range(NBUF)]
            xb16_t = [sp.tile([65, 2048], F16, tag=f"xb16_{i}", name=f"xb16_{i}") for i in range(NBUF)]
            st_t = [sp.tile([128, 1536], F16, tag=f"st{i}", name=f"st{i}") for i in range(NBUF)]
            for i in range(NBUF):
                nc.vector.memset(xb16_t[i][64:65, :], 1.0)

            # v DRAM write view: staging tiles hold 16 full windows
            # (partitions = (e, pix), free = (j, ch); window = 32*nh+16*e+8*s+j)
            # -> per (staging, e): 3-dim AP [[192,64],[12288,8],[1,192]].
            vw = vpix[:].rearrange(
                "(q j) (pix c) -> q pix j c", j=8, c=D,
            )  # [128, 64, 8, 192]

            # ---- phase 2 state: projections, logits, top-4 ------------------
            # emitted in halves: window columns 0..511 depend only on slabs
            # 0..15, so half-0 projections/logits hide under phase 1.
            qta = cp.tile([128, NWIN], F32, tag="qta")
            qtb = cp.tile([64, NWIN], F32, tag="qtb")
            kta = cp.tile([128, NWIN], F32, tag="kta")
            ktb = cp.tile([64, NWIN], F32, tag="ktb")
            lg_t = [cp.tile([128, NWIN], F32, tag=f"lg{i}", name=f"lg{i}")
                    for i in range(8)]
            COPY = mybir.ActivationFunctionType.Identity
            idv = idxf[:].rearrange("(q p t) one -> q p (t one)", p=128, t=TOPK)

            def emit_proj(qc):
                # projection for window-quarter qc (cols 256qc..256qc+255),
                # ready as soon as slab 8*qc+7 has been reduced
                nsl = slice(256 * qc, 256 * (qc + 1))
                for col0, (ta, tb), sc, (ba, bb) in (
                    (0, (qta, qtb), SCALE / SHW, (bqa, bqb)),
                    (192, (kta, ktb), 1.0 / SHW, (bka, bkb)),
                ):
                    for (t_out, d0, dn) in ((ta, 0, 128), (tb, 128, 64)):
                        ps = ppp.tile([dn, 256], F32, tag="pproj", name="ps_proj")
                        nc.tensor.matmul(
                            ps[:], lhsT=wa[:, col0 + d0 : col0 + d0 + dn],
                            rhs=xbar_a[:, nsl], start=True, stop=False,
                        )
                        nc.tensor.matmul(
                            ps[:], lhsT=wb[0:64, col0 + d0 : col0 + d0 + dn],
                            rhs=xbar_b[:, nsl], start=False, stop=True,
                        )
                        nc.scalar.activation(
                            out=t_out[:, nsl], in_=ps[:], func=COPY,
                            bias=ba[:, 0:1] if dn == 128 else bb[:, 0:1], scale=sc,
                        )

            # per-(row-tile, quarter) top-8 partials, computed as soon as a
            # quarter's logits land; the tail only combines + indexes
            qm_t = [cp.tile([128, 32], F32, tag=f"qm{i}", name=f"qm{i}")
                    for i in range(8)]

            def emit_logits(nt, mq):
                # logits row-tile nt vs key-quarter mq
                ps = ppl.tile([128, 256], F32, tag="plog", name="ps_log")
                msl = slice(256 * mq, 256 * (mq + 1))
                nc.tensor.matmul(
                    ps[:], lhsT=qta[:, 128 * nt : 128 * (nt + 1)],
                    rhs=kta[:, msl], start=True, stop=False,
                )
                nc.tensor.matmul(
                    ps[:], lhsT=qtb[:, 128 * nt : 128 * (nt + 1)],
                    rhs=ktb[:, msl], start=False, stop=True,
                )
                if (nt + mq) % 2 == 0:
                    nc.vector.tensor_copy(out=lg_t[nt][:, msl], in_=ps[:])
                else:
                    nc.scalar.copy(out=lg_t[nt][:, msl], in_=ps[:])
                nc.vector.max(out=qm_t[nt][:, 8 * mq : 8 * (mq + 1)], in_=lg_t[nt][:, msl])

            def emit_max(nt):
                lg = lg_t[nt]
                mx8 = p3.tile([128, 8], F32, tag="mx8", name="mx8")
                mi8 = p3.tile([128, 8], U32, tag="mi8", name="mi8")
                # top-8 of the 4 quarter-top-8s == global top-8
                nc.vector.max(out=mx8[:], in_=qm_t[nt][:])
                nc.vector.max_index(out=mi8[:], in_max=mx8[:], in_values=lg[:])
                nc.sync.dma_start(out=idv[nt], in_=mi8[:, 0:TOPK])

            for nh in range(NH):
                xa = xa_t[nh % NBUF]
                xb = xb_t[nh % NBUF]
                xa16 = xa16_t[nh % NBUF]
                xb16 = xb16_t[nh % NBUF]
                nc.scalar.dma_start(out=xa[:], in_=x[0:128, 2048 * nh : 2048 * (nh + 1)])
                nc.scalar.dma_start(out=xb[0:64, :], in_=x[128:192, 2048 * nh : 2048 * (nh + 1)])
                # gpsimd (otherwise idle in phase 1) casts f32->f16 AND
                # reorders to window-major: fp16 column b*128+(e,dh,dw) =
                # pixel (dh,dw) of window nw = 16*e + b.
                xa_wm = xa[:].rearrange("p (dh e b dw) -> p b e dh dw", dh=8, e=2, b=16, dw=8)
                xb_wm = xb[:].rearrange("p (dh e b dw) -> p b e dh dw", dh=8, e=2, b=16, dw=8)
                nc.gpsimd.tensor_copy(out=xa16[:], in_=xa_wm)
                nc.gpsimd.tensor_copy(out=xb16[0:64, :], in_=xb_wm[0:64])
                xar = xa[:].rearrange("p (dh nw dw) -> p nw dh dw", dh=8, nw=32, dw=8)
                xbr = xb[:].rearrange("p (dh nw dw) -> p nw dh dw", dh=8, nw=32, dw=8)
                nc.vector.reduce_sum(
                    out=xbar_a[:, 32 * nh : 32 * (nh + 1)], in_=xar,
                    axis=mybir.AxisListType.XY,
                )
                nc.vector.reduce_sum(
                    out=xbar_b[:, 32 * nh : 32 * (nh + 1)], in_=xbr[0:64],
                    axis=mybir.AxisListType.XY,
                )
                for s in range(2):  # staging tiles: blocks 8s..8s+7
                    st = st_t[(2 * nh + s) % NBUF]
                    for jp in range(4):
                        ps = ppv.tile([128, 384], F32, tag="vps")
                        for sub in range(2):
                            b = 8 * s + 2 * jp + sub
                            o = ps[:, 192 * sub : 192 * (sub + 1)]
                            nc.tensor.matmul(
                                o, lhsT=xa16[:, 128 * b : 128 * (b + 1)],
                                rhs=wv16a[:], start=True, stop=False,
                            )
                            nc.tensor.matmul(
                                o, lhsT=xb16[0:65, 128 * b : 128 * (b + 1)],
                                rhs=wv16b[:], start=False, stop=True,
                            )
                        # psum f32 -> staging f16; alternate DVE/ACT, but
                        # keep DVE clear near the end so the last window-mean
                        # reduces (which gate phase 2) are not queued behind
                        # evac copies
                        if jp % 2 == 0 and nh < 26:
                            nc.vector.tensor_copy(
                                out=st[:, 384 * jp : 384 * (jp + 1)], in_=ps[:]
                            )
                        else:
                            nc.scalar.copy(
                                out=st[:, 384 * jp : 384 * (jp + 1)], in_=ps[:]
                            )
                    # two window-contiguous DMAs: e=0 -> windows 32nh+8s+j,
                    # e=1 -> windows 32nh+16+8s+j (j in 0..8). On the SP ring
                    # (idle in phase 1) so their waits never head-block x loads
                    # or ACT evac copies.
                    for e in range(2):
                        q = 4 * nh + 2 * e + s
                        nc.sync.dma_start(out=vw[q], in_=st[64 * e : 64 * (e + 1), :])

                if nh == 15:
                    # q/k quarters 0-1 + all logits touching only them
                    emit_proj(0)
                    emit_proj(1)
                    for nt in range(4):
                        emit_logits(nt, 0)
                        emit_logits(nt, 1)
                elif nh == 23:
                    # quarter 2: rows 512-767 (nt 4,5) and key cols 512-767
                    emit_proj(2)
                    for nt in range(4):
                        emit_logits(nt, 2)
                    for nt in (4, 5):
                        emit_logits(nt, 0)
                        emit_logits(nt, 1)
                        emit_logits(nt, 2)

            emit_proj(3)
            for nt in range(6):
                emit_logits(nt, 3)
                emit_max(nt)
            for nt in (6, 7):
                for mq in range(4):
                    emit_logits(nt, mq)
                emit_max(nt)

            # ---- phase 3 gather: run the window-id resolution + v gathers ----
            igs = []
            for g in range(8):
                sst = p3.tile([128, 1], U32, tag=f"sst{g}", name=f"sst{g}")
                nc.sync.dma_start(out=sst[:], in_=ssel[128 * g : 128 * (g + 1), :])
                ig = p3.tile([128, 1], U32, tag=f"ig{g}", name=f"ig{g}")
                nc.gpsimd.indirect_dma_start(
                    out=ig[:], out_offset=None, in_=idxf[:],
                    in_offset=IndirectOffsetOnAxis(ap=sst[:, 0:1], axis=0),
                )
                igs.append(ig)
                gt = gp.tile([128, BLK], F16, tag="gt")
                nc.gpsimd.indirect_dma_start(
                    out=gt[:], out_offset=None, in_=vpix[:],
                    in_offset=IndirectOffsetOnAxis(ap=ig[:, 0:1], axis=0),
                )
                nc.scalar.dma_start(out=out[128 * g : 128 * (g + 1), :], in_=gt[:])

    return nc


def make_in_maps(x_full, w_qkv, b_qkv):
    """x_full (2, 192, 256, 256) -> per-core input dicts."""
    ins = []
    for core in range(8):
        b = core // 4
        q = core % 4
        ins.append(
            {
                "x": np.ascontiguousarray(x_full[b].reshape(C, H * W)),
                "wqkv": np.ascontiguousarray(w_qkv),
                "bqkv": np.ascontiguousarray(b_qkv.reshape(576, 1)),
                "ssel": np.arange(1024 * q, 1024 * (q + 1), dtype=np.uint32).reshape(
                    NWIN, 1
                ),
            }
        )
    return ins


def assemble(results):
    """per-core 'out' (1024, 12288) -> (2, 1024, 4, 64, 192)."""
    full = np.empty((2, NWIN, TOPK, SHW, D), dtype=np.float32)
    for core in range(8):
        b = core // 4
        q = core % 4
        r = results[core]["out"].astype(np.float32).reshape(256, TOPK, SHW, D)
        full[b, 256 * q : 256 * (q + 1)] = r
    return full


_NC_CACHE = None


def _get_nc():
    global _NC_CACHE
    if _NC_CACHE is None:
        _NC_CACHE = build_nc()
    return _NC_CACHE


def kernel(x, w_qkv, b_qkv):
    from concourse.bass_utils import run_bass_kernel_spmd

    x = np.ascontiguousarray(np.asarray(x, dtype=np.float32))
    w_qkv = np.ascontiguousarray(np.asarray(w_qkv, dtype=np.float32))
    b_qkv = np.ascontiguousarray(np.asarray(b_qkv, dtype=np.float32))

    nc = _get_nc()
    in_maps = make_in_maps(x, w_qkv, b_qkv)
    res = run_bass_kernel_spmd(nc, in_maps, core_ids=list(range(8)))
    return assemble(res.results)

